# revision 8
# baseline (speedup 1.0000x reference)
"""Trainium2 Bass kernel for a causal attention head block (B=4, T=2048, C=2048,
H=16, D=128) with RoPE (single fixed position, folded into weights on host).

Sharding: 8 cores = 4 batches x 2 head-groups (8 heads each).

v2: fully SBUF-resident (no DRAM spills of q/k/v), masked-diagonal column
truncation in both the scores and AV matmuls, and PE backfill during the
ACT-bound attention phase: the last three heads' QK projections (q4..k6) and
the first out-projection token tiles are interleaved into the attention slot
pipeline so the PE never waits for exp.

Engine budget per core (target ~570us): PE 2720 matmuls ~567us; ACT (exp +
copies) ~210us; DVE ~150us -- PE-bound throughout.
"""
import numpy as np

B, T, C, H, D = 4, 2048, 2048, 16, 128
ROPE_BASE = 10000.0
HG = H // 2            # heads per core: 8
JQ = HG * D            # 1024 q (or k, or v) channels per core
NCORES = 8
NCT = C // 128         # 16 contraction tiles
NTT = T // 128         # 16 token tiles
NTC = T // 512         # 4 token chunks of 512

_CACHE = {}

# B-phase head processing order and the reserve QK j-tiles backfilled into
# each head's slot pipeline (head 4 runs last; its backfill is phase C).
B_ORDER = (0, 1, 2, 3, 7, 6, 5, 4)
RESERVE = {0: (('q', 7, (0, 1, 2, 3)),),
           1: (('k', 7, (0, 1, 2, 3)),),
           2: (('q', 6, (0, 1, 2, 3)),),
           3: (('k', 6, (0, 1, 2, 3)),),
           7: (('q', 5, (0, 1, 2, 3)),),
           6: (('k', 5, (0, 1, 2, 3)),),
           5: (('q', 4, (0, 1, 2, 3)), ('k', 4, (0, 1, 2, 3)))}


def _build_nc():
    import concourse.bass as bass
    import concourse.mybir as mybir
    import concourse.tile as tile
    from concourse import bacc

    f32 = mybir.dt.float32
    f16, bf16 = mybir.dt.float16, mybir.dt.bfloat16
    ds, ts = bass.ds, bass.ts
    Exp = mybir.ActivationFunctionType.Exp
    Ident = mybir.ActivationFunctionType.Identity
    mult = mybir.AluOpType.mult
    add = mybir.AluOpType.add

    nc = bacc.Bacc("TRN2", target_bir_lowering=False, debug=False)
    xT = nc.dram_tensor("xT", [C, T], bf16, kind="ExternalInput").ap()
    WqkT = nc.dram_tensor("WqkT", [C, 2 * JQ], bf16, kind="ExternalInput").ap()
    WvT = nc.dram_tensor("WvT", [C, JQ], bf16, kind="ExternalInput").ap()
    WoT = nc.dram_tensor("WoT", [JQ, C], bf16, kind="ExternalInput").ap()
    bq = nc.dram_tensor("bq", [JQ, 1], f32, kind="ExternalInput").ap()
    # msk[:, 0:128] = lower triangle (p <= i), msk[:, 128:256] = ones
    msk = nc.dram_tensor("msk", [128, 256], f16, kind="ExternalInput").ap()
    o = nc.dram_tensor("o", [T, C], bf16, kind="ExternalOutput").ap()

    with tile.TileContext(nc) as tc:
        with tc.tile_pool(name="const", bufs=1) as cpool:
            # const tiles are DMA'd after the phase-A input loads (below) so
            # they don't block the gpsimd ring ahead of the first V chains
            mask_t = cpool.tile([128, 256], f16, tag="mask")
            tri = mask_t[:, 0:128]
            ones_sq = mask_t[:, 128:256]
            # warm-up scratch: zeroed source for dummy matmuls that keep the
            # PE busy (and the HAM clock gate warm) during the DMA-bound start
            wsrc = cpool.tile([128, 512], bf16, tag="wsrc")
            nc.gpsimd.memset(wsrc[:], 0.0)
            bq_t = []
            for j in range(JQ // 128):
                t_ = cpool.tile([128, 1], f32, tag=f"bq{j}")
                bq_t.append(t_)

            # persistent SBUF-resident tensors (qk pool alloc'd post A-V)
            vpool_cm = tc.tile_pool(name="vres", bufs=1)
            vpool = vpool_cm.__enter__()
            V_t = [vpool.tile([128, JQ], f16, tag=f"V{tt}", name=f"V{tt}")
                   for tt in range(NTT)]

            # right-side pools: x + weight streaming (released before/at B)
            xpool_cm = tc.tile_pool(name="xt", bufs=1, side="right")
            xpool = xpool_cm.__enter__()
            wpool_cm = tc.tile_pool(name="wqk", bufs=1, side="right")
            wpool = wpool_cm.__enter__()

            # ---------------- Phase A-V ----------------
            xt = [None] * NCT
            wts_q03 = []
            with tc.tile_pool(name="wv", bufs=33, side="right") as wvpool, \
                 tc.tile_pool(name="psV", bufs=8, space="PSUM") as psvpool:
                # alternate DMA queues (sync/gpsimd) so the two rings issue
                # input loads in parallel -- the A-V start is DMA-bound
                qs = [nc.sync, nc.sync]
                wvs_all = [[], []]
                for ci in range(NCT):
                    t_ = xpool.tile([128, T], bf16, tag=f"x{ci}",
                                    name=f"x{ci}")
                    # first token-column block right away so the tt=0
                    # accumulation chain unblocks after ~2.6 MB of DMA
                    qs[ci % 2].dma_start(t_[:, 0:128], xT[ts(ci, 128), 0:128])
                    xt[ci] = t_
                    w_ = wvpool.tile([128, 512], bf16, tag="wv",
                                     name=f"wv0_{ci}")
                    qs[(ci + 1) % 2].dma_start(
                        w_[:], WvT[ts(ci, 128), ds(0, 512)])
                    wvs_all[0].append(w_)
                for ci in range(NCT):
                    qs[ci % 2].dma_start(xt[ci][:, 128:512],
                                         xT[ts(ci, 128), 128:512])
                for tcol in range(1, NTC):
                    for ci in range(NCT):
                        qs[ci % 2].dma_start(
                            xt[ci][:, ts(tcol, 512)],
                            xT[ts(ci, 128), ts(tcol, 512)])
                # second-chunk V weights + first QK group prefetch up front
                for ci in range(NCT):
                    w_ = wvpool.tile([128, 512], bf16, tag="wv",
                                     name=f"wv1_{ci}")
                    qs[ci % 2].dma_start(
                        w_[:], WvT[ts(ci, 128), ds(512, 512)])
                    wvs_all[1].append(w_)
                for ci in range(NCT):
                    w_ = wpool.tile([128, 512], bf16, tag="wg",
                                    bufs=32, name=f"wq03_{ci}")
                    qs[ci % 2].dma_start(
                        w_[:], WqkT[ts(ci, 128), ds(0, 512)])
                    wts_q03.append(w_)
                nc.sync.dma_start(mask_t[:], msk[:])
                for j in range(JQ // 128):
                    nc.sync.dma_start(bq_t[j][:], bq[ts(j, 128), :])
                # dummy matmuls on the scratch tile: write-only psum bank,
                # never read -- pure PE-warming while input DMAs land
                wps = psvpool.tile([128, 512], f32, tag="warm", bufs=1)

                def warm():
                    nc.tensor.matmul(wps[:], wsrc[:, 0:128], wsrc[:],
                                     start=True, stop=True)

                for _ in range(4):
                    warm()
                for vch in range(2):
                    wvs = wvs_all[vch]
                    for tt in range(NTT):
                        ps = psvpool.tile([128, 512], f32, tag="psv",
                                          bufs=7)
                        for ci in range(NCT):
                            if vch == 0 and (
                                    tt == 0 or (tt == 1 and ci % 2 == 0)
                                    or (tt == 2 and ci % 4 == 0)):
                                warm()
                            nc.tensor.matmul(
                                ps[:], xt[ci][:, ts(tt, 128)], wvs[ci][:],
                                start=(ci == 0), stop=(ci == NCT - 1))
                        nc.vector.tensor_copy(
                            V_t[tt][:, ds(vch * 512, 512)], ps[:])

            # ---------------- Phase A-QK (main) ----------------
            # j-tiles q0..q3, k0..k3 (group-loaded);
            # q4..q7, k4..k7 are reserved for B-phase backfill.
            qkpool_cm = tc.tile_pool(name="qkres", bufs=1)
            qkpool = qkpool_cm.__enter__()
            # 0..7 = q heads, 8..15 = k heads, layout [j(128), T]
            QK_t = [qkpool.tile([128, T], bf16, tag=f"J{j}", name=f"J{j}")
                    for j in range(16)]
            with tc.tile_pool(name="psA", bufs=4, space="PSUM") as pspool:
                wts_k03 = []
                for ci in range(NCT):
                    w_ = wpool.tile([128, 512], bf16, tag="wg", bufs=32,
                                    name=f"wk03_{ci}")
                    qs[ci % 2].dma_start(
                        w_[:], WqkT[ts(ci, 128), ds(JQ, 512)])
                    wts_k03.append(w_)

                def qk_jtile(qk, h, wts, wsl, split=False):
                    """emit one [128, T] projection j-tile (64 MMs+4 copies)"""
                    jt_idx = h if qk == 'q' else 8 + h
                    pss_l = [pspool.tile([128, 512], f32, tag="psa",
                                         name=f"psa{jt_idx}_{t2}")
                             for t2 in range(NTC)]
                    for ci in range(NCT):
                        for tch in range(NTC):
                            nc.tensor.matmul(
                                pss_l[tch][:], wts[ci][:, wsl],
                                xt[ci][:, ts(tch, 512)],
                                start=(ci == 0), stop=(ci == NCT - 1))
                    for tch in range(NTC):
                        if qk == 'q':
                            nc.scalar.activation(
                                QK_t[h][:, ts(tch, 512)], pss_l[tch][:],
                                Ident, bias=bq_t[h][:, 0:1])
                        elif split and tch >= 2:
                            # split the final j-tile's copies across engines
                            # so the B-phase isn't gated on one queue
                            nc.scalar.copy(
                                QK_t[8 + h][:, ts(tch, 512)], pss_l[tch][:])
                        else:
                            nc.vector.tensor_copy(
                                QK_t[8 + h][:, ts(tch, 512)], pss_l[tch][:])

                for h in range(4):
                    qk_jtile('q', h, wts_q03, ds(h * 128, 128))
                    qk_jtile('k', h, wts_k03, ds(h * 128, 128),
                             split=(h == 3))
            wpool_cm.__exit__(None, None, None)

            # ---------------- Phase B + interleaves ----------------
            ypool_cm = tc.tile_pool(name="ysb", bufs=1)
            ypool = ypool_cm.__enter__()
            y_t = [ypool.tile([128, T], bf16, tag=f"y{h}", name=f"y{h}")
                   for h in range(HG)]
            # reserve-weight piece pool (right side, small)
            wrpool_cm = tc.tile_pool(name="wres", bufs=1, side="right")
            wrpool = wrpool_cm.__enter__()

            espool_cm = tc.tile_pool(name="es", bufs=4)
            espool = espool_cm.__enter__()
            accpool_cm = tc.tile_pool(name="acc", bufs=2)
            accpool = accpool_cm.__enter__()
            npool_cm = tc.tile_pool(name="nrm", bufs=2)
            npool = npool_cm.__enter__()
            pss_cm = tc.tile_pool(name="psS", bufs=3, space="PSUM")
            pss = pss_cm.__enter__()
            psy_cm = tc.tile_pool(name="psY", bufs=2, space="PSUM")
            psy = psy_cm.__enter__()
            psd_cm = tc.tile_pool(name="psD", bufs=1, space="PSUM")
            psd = psd_cm.__enter__()
            aqk_cm = tc.tile_pool(name="psR", bufs=2, space="PSUM")
            aqkps = aqk_cm.__enter__()
            psoB_cm = [None]
            psoB = [None]

            wo_t = [None] * HG
            ospool = [None]   # allocated after x releases

            # --- reserve QK backfill thunks -------------------------------
            def reserve_thunks(head):
                """PE thunk list for this head's reserve j-tiles; DMAs are
                emitted immediately (they land well before use)."""
                thunks = []
                for qk, h2, tchs in RESERVE.get(head, ()):
                    jcol = h2 * 128 if qk == 'q' else JQ + h2 * 128
                    wr = []
                    for ci in range(NCT):
                        wr_ = wrpool.tile([128, 128], bf16, tag=f"wr{ci}",
                                          bufs=1, name=f"wr_{qk}{h2}_{ci}")
                        nc.sync.dma_start(
                            wr_[:], WqkT[ts(ci, 128), ds(jcol, 128)])
                        wr.append(wr_)
                    for tch in tchs:
                        def mk_chain(qk=qk, h2=h2, tch=tch, wr=wr):
                            ps_r = aqkps.tile([128, 512], f32, tag="pr",
                                              name="pr")
                            def mm(ci, ps_r=ps_r, qk=qk, h2=h2, tch=tch,
                                   wr=wr):
                                nc.tensor.matmul(
                                    ps_r[:], wr[ci][:],
                                    xt[ci][:, ts(tch, 512)],
                                    start=(ci == 0), stop=(ci == NCT - 1))
                                if ci == NCT - 1:
                                    if qk == 'q':
                                        nc.scalar.activation(
                                            QK_t[h2][:, ts(tch, 512)],
                                            ps_r[:], Ident,
                                            bias=bq_t[h2][:, 0:1])
                                    else:
                                        nc.vector.tensor_copy(
                                            QK_t[8 + h2][:, ts(tch, 512)],
                                            ps_r[:])
                            return [lambda ci=ci: mm(ci)
                                    for ci in range(NCT)]
                        thunks.append(mk_chain)
                # flatten lazily: list of chain-makers -> iterator of MMs
                def gen():
                    for mk in thunks:
                        for fn in mk():
                            yield fn
                return gen()

            # --- phase C chains -------------------------------------------
            c_emitted = [0]     # count of (tt, cch) chains already emitted

            def c_chain(idx, pool):
                tt, cch = divmod(idx, NTC)
                po = pool.tile([128, 512], f32, tag="po", name=f"po{idx}")
                def mm(ch, po=po, tt=tt, cch=cch, idx=idx):
                    nc.tensor.matmul(
                        po[:], y_t[ch][:, ts(tt, 128)],
                        wo_t[ch][:, ds(cch * 512, 512)],
                        start=(ch == 0), stop=(ch == HG - 1))
                    if ch == HG - 1:
                        st = ospool[0].tile([128, 512], bf16, tag="ost")
                        if idx % 2 == 0:
                            nc.vector.tensor_copy(st[:], po[:])
                        else:
                            nc.scalar.copy(st[:], po[:])
                        nc.sync.dma_start(
                            o[ts(tt, 128), ds(cch * 512, 512)], st[:])
                return [lambda ch=ch: mm(ch) for ch in range(HG)]

            def c_thunks(max_chains):
                def gen():
                    n = 0
                    while n < max_chains and c_emitted[0] < NTT * NTC:
                        idx = c_emitted[0]
                        c_emitted[0] += 1
                        n += 1
                        for fn in c_chain(idx, psoB[0]):
                            yield fn
                return gen()

            # --- phase B slot pipeline ------------------------------------
            LAG = 3
            for bi, h in enumerate(B_ORDER):
                if h == 4:
                    # head 4 is last: backfill with early phase-C chains
                    # (chunk-0 y of all heads is complete after chunk 0)
                    backfill = None     # armed per-chunk below
                else:
                    backfill = reserve_thunks(h)
                kt = QK_t[8 + h]
                avq = []        # (due_slot, es_ap, psY_ap, v_ap, start, stop)
                events = {}     # slot -> [thunk]
                credit = [0.0]
                if backfill is not None:
                    n_res = sum(len(e[2]) * NCT for e in RESERVE.get(h, ()))
                    rate = n_res / 38.0 + 0.1
                else:
                    rate = 1.2
                backref = [backfill]

                def at_slot(s, fn):
                    events.setdefault(s, []).append(fn)

                def pump(slot):
                    for fn in events.pop(slot, []):
                        fn()
                    while avq and avq[0][0] <= slot:
                        _, e_, py_, vs_, fi_, la_ = avq.pop(0)
                        nc.tensor.matmul(py_, vs_, e_, start=fi_,
                                         stop=la_)
                    credit[0] += rate
                    while credit[0] >= 1.0 and backref[0] is not None:
                        fn = next(backref[0], None)
                        if fn is None:
                            backref[0] = None
                            break
                        fn()
                        credit[0] -= 1.0

                def mk_dens(acc_, st):
                    def fn():
                        ps_d = psd.tile([128, 512], f32, tag="pd", name="pd")
                        nc.tensor.matmul(ps_d[:], ones_sq, acc_[:],
                                         start=True, stop=True)
                        st["ps_d"] = ps_d
                    return fn

                def mk_rec(st):
                    def fn():
                        rb = npool.tile([128, 512], f32, tag="rb", name="rb")
                        nc.vector.reciprocal_approx_fast(rb[:], st["ps_d"][:])
                        st["rb"] = rb
                    return fn

                def mk_mult(ps_y_, h_, ci_, st):
                    def fn():
                        nc.vector.tensor_tensor(
                            y_t[h_][:, ds(ci_ * 512, 512)], ps_y_[:],
                            st["rb"][:], mult)
                    return fn

                slot = 0
                for ci in range(NTC):
                    ps_y = psy.tile([128, 512], f32, tag="py")
                    acc = accpool.tile([128, 512], f16, tag="acc", name="acc")
                    njt = 4 * (ci + 1)
                    # order: a full-width tile first (jt=4ci, off=0), then
                    # spread the remaining diagonal tiles (短 exp chains)
                    # through the plain-tile interior
                    plain = list(range(4 * ci))
                    diag = list(range(4 * ci + 1, njt))
                    order = [4 * ci]
                    if plain:
                        step = max(1, len(plain) // 3)
                        pi = 0
                        for dtile in diag:
                            order.extend(plain[pi:pi + step])
                            pi += step
                            order.append(dtile)
                        order.extend(plain[pi:])
                    else:
                        order.extend(diag)
                    for oi_, jt in enumerate(order):
                        off = jt * 128 - ci * 512
                        w = 512 - max(off, 0)
                        ps_s = pss.tile([128, 512], f32, tag="pss")
                        nc.tensor.matmul(
                            ps_s[:, ds(512 - w, w)], kt[:, ts(jt, 128)],
                            QK_t[h][:, ds(ci * 512 + 512 - w, w)],
                            start=True, stop=True)
                        pump(slot)
                        es = espool.tile([128, 512], f16, tag="es")
                        nc.scalar.activation(es[:, ds(512 - w, w)],
                                             ps_s[:, ds(512 - w, w)], Exp)
                        if off >= 0:
                            nc.vector.tensor_tensor(
                                es[:, ds(off, 128)], es[:, ds(off, 128)],
                                tri[:, 0:128], mult)
                        if oi_ == 0:
                            nc.vector.tensor_copy(acc[:], es[:])
                        else:
                            nc.vector.tensor_tensor(
                                acc[:, ds(512 - w, w)], acc[:, ds(512 - w, w)],
                                es[:, ds(512 - w, w)], add)
                        avq.append((slot + LAG, es[:, ds(512 - w, w)],
                                    ps_y[:, ds(512 - w, w)],
                                    V_t[jt][:, ds(h * 128, 128)],
                                    oi_ == 0, oi_ == njt - 1))
                        slot += 1
                    st = {}
                    at_slot(slot + 1, mk_dens(acc, st))
                    at_slot(slot + 2, mk_rec(st))
                    at_slot(slot + 4, mk_mult(ps_y, h, ci, st))
                    if h == 4:
                        # arm phase-C backfill once chunk-0 y is final
                        def arm():
                            if backref[0] is None:
                                credit[0] = 0.0
                                backref[0] = c_thunks(6)
                        if ci == 0:
                            at_slot(slot + 5, arm)
                # drain the pipeline and flush any leftover backfill
                while avq or events:
                    pump(slot)
                    slot += 1
                if backref[0] is not None:
                    for fn in backref[0]:
                        fn()
                    backref[0] = None

                if bi == len(B_ORDER) - 2:
                    # all reserve QK done (after B(5)): release x + wres,
                    # load the out-projection weights into the freed space
                    wrpool_cm.__exit__(None, None, None)
                    xpool_cm.__exit__(None, None, None)
                    aqk_cm.__exit__(None, None, None)
                    psoB_cm[0] = tc.tile_pool(name="psoB", bufs=2,
                                              space="PSUM")
                    psoB[0] = psoB_cm[0].__enter__()
                    wopool_cm = tc.tile_pool(name="woc", bufs=1,
                                             side="right")
                    wopool = wopool_cm.__enter__()
                    for ch in range(HG):
                        w_ = wopool.tile([128, C], bf16, tag=f"wo{ch}",
                                         name=f"wo{ch}")
                        nc.sync.dma_start(w_[:], WoT[ts(ch, 128), :])
                        wo_t[ch] = w_
                    osp_cm = tc.tile_pool(name="ost", bufs=6)
                    ospool[0] = osp_cm.__enter__()

            # ---------------- Phase C (remaining) ----------------
            psoB_cm[0].__exit__(None, None, None)
            psd_cm.__exit__(None, None, None)
            psy_cm.__exit__(None, None, None)
            pss_cm.__exit__(None, None, None)
            with tc.tile_pool(name="psO", bufs=4, space="PSUM") as psoC:
                while c_emitted[0] < NTT * NTC:
                    idx = c_emitted[0]
                    c_emitted[0] += 1
                    for fn in c_chain(idx, psoC):
                        fn()
            # release in LIFO order
            osp_cm.__exit__(None, None, None)
            wopool_cm.__exit__(None, None, None)
            npool_cm.__exit__(None, None, None)
            accpool_cm.__exit__(None, None, None)
            espool_cm.__exit__(None, None, None)
            ypool_cm.__exit__(None, None, None)
            qkpool_cm.__exit__(None, None, None)
            vpool_cm.__exit__(None, None, None)
    nc.compile()
    return nc


def _rope_matrix():
    inv_freq = 1.0 / (ROPE_BASE ** (np.arange(0, D, 2, dtype=np.float64) / D))
    freqs = np.float64(T) * inv_freq
    emb = np.concatenate([freqs, freqs])
    cos, sin = np.cos(emb), np.sin(emb)
    R = np.zeros((D, D))
    for j in range(D):
        R[j, j] = cos[j]
    for j in range(64):
        R[j, 2 * j + 1] += -sin[j]
    for j in range(64, 128):
        R[j, 2 * (j - 64)] += sin[j]
    return R


def _host_mask():
    m = np.zeros((128, 256), dtype=np.float16)
    jj = np.arange(128)
    m[:, 0:128] = (jj[:, None] <= jj[None, :])
    m[:, 128:256] = 1.0
    return m


def kernel(x, Wqkv, bqkv, Wout, bout):
    import ml_dtypes
    from concourse.bass_utils import run_bass_kernel_spmd

    bfloat16 = ml_dtypes.bfloat16

    if "nc" not in _CACHE:
        _CACHE["nc"] = _build_nc()
    nc = _CACHE["nc"]

    x = np.asarray(x, dtype=np.float32)
    Wqkv64 = np.asarray(Wqkv, dtype=np.float64)
    bqkv64 = np.asarray(bqkv, dtype=np.float64)
    Wout64 = np.asarray(Wout, dtype=np.float64)
    bout64 = np.asarray(bout, dtype=np.float64)

    R = _rope_matrix()
    scale = 1.0 / np.sqrt(np.float64(D))
    Wq = Wqkv64[:C].reshape(H, D, C)
    Wk = Wqkv64[C:2 * C].reshape(H, D, C)
    Wv = Wqkv64[2 * C:].reshape(H, D, C)
    bqv = bqkv64[:C].reshape(H, D)
    bv = bqkv64[2 * C:]

    Wq_f = np.einsum('jk,hkc->hjc', R, Wq) * scale
    bq_f = np.einsum('jk,hk->hj', R, bqv) * scale
    Wk_f = np.einsum('jk,hkc->hjc', R, Wk)
    bias_final = (bout64 + Wout64 @ bv).astype(np.float32)

    mask = _host_mask()
    in_maps = []
    xTb = [np.ascontiguousarray(x[b].T).astype(bfloat16) for b in range(B)]
    shard = {}
    for hg in range(2):
        hs = slice(hg * HG, (hg + 1) * HG)
        wqk = np.concatenate(
            [Wq_f[hs].reshape(JQ, C), Wk_f[hs].reshape(JQ, C)], axis=0)
        shard[hg] = dict(
            WqkT=np.ascontiguousarray(wqk.T).astype(bfloat16),
            WvT=np.ascontiguousarray(
                Wv[hs].reshape(JQ, C).T).astype(bfloat16),
            WoT=np.ascontiguousarray(
                Wout64[:, hg * JQ:(hg + 1) * JQ].T).astype(bfloat16),
            bq=bq_f[hs].reshape(JQ, 1).astype(np.float32),
        )
    for core in range(NCORES):
        b, hg = core // 2, core % 2
        in_maps.append(dict(xT=xTb[b], msk=mask, **shard[hg]))

    res = run_bass_kernel_spmd(nc, in_maps, core_ids=list(range(NCORES)),
                               **_CACHE.get("run_kwargs", {}))
    _CACHE["last_result"] = res
    out = np.empty((B, T, C), dtype=np.float32)
    for b in range(B):
        out[b] = (res.results[2 * b]["o"].astype(np.float32)
                  + res.results[2 * b + 1]["o"].astype(np.float32)
                  + bias_final)
    return out


# revision 9
# speedup vs baseline: 1.0032x; 1.0032x over previous
"""Trainium2 Bass kernel for a causal attention head block (B=4, T=2048, C=2048,
H=16, D=128) with RoPE (single fixed position, folded into weights on host).

Sharding: 8 cores = 4 batches x 2 head-groups (8 heads each).

v2: fully SBUF-resident (no DRAM spills of q/k/v), masked-diagonal column
truncation in both the scores and AV matmuls, and PE backfill during the
ACT-bound attention phase: the last three heads' QK projections (q4..k6) and
the first out-projection token tiles are interleaved into the attention slot
pipeline so the PE never waits for exp.

Engine budget per core (target ~570us): PE 2720 matmuls ~567us; ACT (exp +
copies) ~210us; DVE ~150us -- PE-bound throughout.
"""
import numpy as np

B, T, C, H, D = 4, 2048, 2048, 16, 128
ROPE_BASE = 10000.0
HG = H // 2            # heads per core: 8
JQ = HG * D            # 1024 q (or k, or v) channels per core
NCORES = 8
NCT = C // 128         # 16 contraction tiles
NTT = T // 128         # 16 token tiles
NTC = T // 512         # 4 token chunks of 512

_CACHE = {}

# B-phase head processing order and the reserve QK j-tiles backfilled into
# each head's slot pipeline (head 4 runs last; its backfill is phase C).
B_ORDER = (0, 1, 2, 3, 7, 6, 5, 4)
RESERVE = {0: (('q', 7, (0, 1, 2, 3)),),
           1: (('k', 7, (0, 1, 2, 3)),),
           2: (('q', 6, (0, 1, 2, 3)),),
           3: (('k', 6, (0, 1, 2, 3)),),
           7: (('q', 5, (0, 1, 2, 3)),),
           6: (('k', 5, (0, 1, 2, 3)),),
           5: (('q', 4, (0, 1, 2, 3)), ('k', 4, (0, 1, 2, 3)))}


def _build_nc():
    import concourse.bass as bass
    import concourse.mybir as mybir
    import concourse.tile as tile
    from concourse import bacc

    f32 = mybir.dt.float32
    f16, bf16 = mybir.dt.float16, mybir.dt.bfloat16
    ds, ts = bass.ds, bass.ts
    Exp = mybir.ActivationFunctionType.Exp
    Ident = mybir.ActivationFunctionType.Identity
    mult = mybir.AluOpType.mult
    add = mybir.AluOpType.add

    nc = bacc.Bacc("TRN2", target_bir_lowering=False, debug=False)
    xT = nc.dram_tensor("xT", [C, T], bf16, kind="ExternalInput").ap()
    WqkT = nc.dram_tensor("WqkT", [C, 2 * JQ], bf16, kind="ExternalInput").ap()
    WvT = nc.dram_tensor("WvT", [C, JQ], bf16, kind="ExternalInput").ap()
    WoT = nc.dram_tensor("WoT", [JQ, C], bf16, kind="ExternalInput").ap()
    bq = nc.dram_tensor("bq", [JQ, 1], f32, kind="ExternalInput").ap()
    # msk[:, 0:128] = lower triangle (p <= i), msk[:, 128:256] = ones
    msk = nc.dram_tensor("msk", [128, 256], f16, kind="ExternalInput").ap()
    o = nc.dram_tensor("o", [T, C], bf16, kind="ExternalOutput").ap()

    with tile.TileContext(nc) as tc:
        with tc.tile_pool(name="const", bufs=1) as cpool:
            # const tiles are DMA'd after the phase-A input loads (below) so
            # they don't block the gpsimd ring ahead of the first V chains
            mask_t = cpool.tile([128, 256], f16, tag="mask")
            tri = mask_t[:, 0:128]
            ones_sq = mask_t[:, 128:256]
            # warm-up scratch: zeroed source for dummy matmuls that keep the
            # PE busy (and the HAM clock gate warm) during the DMA-bound start
            wsrc = cpool.tile([128, 512], bf16, tag="wsrc")
            nc.gpsimd.memset(wsrc[:], 0.0)
            bq_t = []
            for j in range(JQ // 128):
                t_ = cpool.tile([128, 1], f32, tag=f"bq{j}")
                bq_t.append(t_)

            # persistent SBUF-resident tensors (qk pool alloc'd post A-V)
            vpool_cm = tc.tile_pool(name="vres", bufs=1)
            vpool = vpool_cm.__enter__()
            V_t = [vpool.tile([128, JQ], f16, tag=f"V{tt}", name=f"V{tt}")
                   for tt in range(NTT)]

            # right-side pools: x + weight streaming (released before/at B)
            xpool_cm = tc.tile_pool(name="xt", bufs=1, side="right")
            xpool = xpool_cm.__enter__()
            wpool_cm = tc.tile_pool(name="wqk", bufs=1, side="right")
            wpool = wpool_cm.__enter__()

            # ---------------- Phase A-V ----------------
            xt = [None] * NCT
            wts_q03 = []
            with tc.tile_pool(name="wv", bufs=33, side="right") as wvpool, \
                 tc.tile_pool(name="psV", bufs=8, space="PSUM") as psvpool:
                # alternate DMA queues (sync/gpsimd) so the two rings issue
                # input loads in parallel -- the A-V start is DMA-bound
                qs = [nc.sync, nc.sync]
                wvs_all = [[], []]
                for ci in range(NCT):
                    t_ = xpool.tile([128, T], bf16, tag=f"x{ci}",
                                    name=f"x{ci}")
                    # first token-column block right away so the tt=0
                    # accumulation chain unblocks after ~2.6 MB of DMA
                    qs[ci % 2].dma_start(t_[:, 0:128], xT[ts(ci, 128), 0:128])
                    xt[ci] = t_
                    w_ = wvpool.tile([128, 512], bf16, tag="wv",
                                     name=f"wv0_{ci}")
                    qs[(ci + 1) % 2].dma_start(
                        w_[:], WvT[ts(ci, 128), ds(0, 512)])
                    wvs_all[0].append(w_)
                for ci in range(NCT):
                    qs[ci % 2].dma_start(xt[ci][:, 128:512],
                                         xT[ts(ci, 128), 128:512])
                for tcol in range(1, NTC):
                    for ci in range(NCT):
                        qs[ci % 2].dma_start(
                            xt[ci][:, ts(tcol, 512)],
                            xT[ts(ci, 128), ts(tcol, 512)])
                # second-chunk V weights + first QK group prefetch up front
                for ci in range(NCT):
                    w_ = wvpool.tile([128, 512], bf16, tag="wv",
                                     name=f"wv1_{ci}")
                    qs[ci % 2].dma_start(
                        w_[:], WvT[ts(ci, 128), ds(512, 512)])
                    wvs_all[1].append(w_)
                for ci in range(NCT):
                    w_ = wpool.tile([128, 512], bf16, tag="wg",
                                    bufs=32, name=f"wq03_{ci}")
                    qs[ci % 2].dma_start(
                        w_[:], WqkT[ts(ci, 128), ds(0, 512)])
                    wts_q03.append(w_)
                nc.sync.dma_start(mask_t[:], msk[:])
                for j in range(JQ // 128):
                    nc.sync.dma_start(bq_t[j][:], bq[ts(j, 128), :])
                # dummy matmuls on the scratch tile: write-only psum bank,
                # never read -- pure PE-warming while input DMAs land
                wps = psvpool.tile([128, 512], f32, tag="warm", bufs=1)

                def warm():
                    nc.tensor.matmul(wps[:], wsrc[:, 0:128], wsrc[:],
                                     start=True, stop=True)

                for _ in range(4):
                    warm()
                for vch in range(2):
                    wvs = wvs_all[vch]
                    for tt in range(NTT):
                        ps = psvpool.tile([128, 512], f32, tag="psv",
                                          bufs=7)
                        for ci in range(NCT):
                            if vch == 0 and (
                                    tt == 0 or (tt == 1 and ci % 2 == 0)
                                    or (tt == 2 and ci % 4 == 0)):
                                warm()
                            nc.tensor.matmul(
                                ps[:], xt[ci][:, ts(tt, 128)], wvs[ci][:],
                                start=(ci == 0), stop=(ci == NCT - 1))
                        nc.vector.tensor_copy(
                            V_t[tt][:, ds(vch * 512, 512)], ps[:])

            # ---------------- Phase A-QK (main) ----------------
            # j-tiles q0..q3, k0..k3 (group-loaded);
            # q4..q7, k4..k7 are reserved for B-phase backfill.
            qkpool_cm = tc.tile_pool(name="qkres", bufs=1)
            qkpool = qkpool_cm.__enter__()
            # 0..7 = q heads, 8..15 = k heads, layout [j(128), T]
            QK_t = [qkpool.tile([128, T], bf16, tag=f"J{j}", name=f"J{j}")
                    for j in range(16)]
            with tc.tile_pool(name="psA", bufs=4, space="PSUM") as pspool:
                wts_k03 = []
                for ci in range(NCT):
                    w_ = wpool.tile([128, 512], bf16, tag="wg", bufs=32,
                                    name=f"wk03_{ci}")
                    qs[ci % 2].dma_start(
                        w_[:], WqkT[ts(ci, 128), ds(JQ, 512)])
                    wts_k03.append(w_)

                def qk_jtile(qk, h, wts, wsl, split=False):
                    """emit one [128, T] projection j-tile (64 MMs+4 copies)"""
                    jt_idx = h if qk == 'q' else 8 + h
                    pss_l = [pspool.tile([128, 512], f32, tag="psa",
                                         name=f"psa{jt_idx}_{t2}")
                             for t2 in range(NTC)]
                    for ci in range(NCT):
                        for tch in range(NTC):
                            nc.tensor.matmul(
                                pss_l[tch][:], wts[ci][:, wsl],
                                xt[ci][:, ts(tch, 512)],
                                start=(ci == 0), stop=(ci == NCT - 1))
                    for tch in range(NTC):
                        if qk == 'q':
                            nc.scalar.activation(
                                QK_t[h][:, ts(tch, 512)], pss_l[tch][:],
                                Ident, bias=bq_t[h][:, 0:1])
                        elif split and tch >= 2:
                            # split the final j-tile's copies across engines
                            # so the B-phase isn't gated on one queue
                            nc.scalar.copy(
                                QK_t[8 + h][:, ts(tch, 512)], pss_l[tch][:])
                        else:
                            nc.vector.tensor_copy(
                                QK_t[8 + h][:, ts(tch, 512)], pss_l[tch][:])

                for h in range(4):
                    qk_jtile('q', h, wts_q03, ds(h * 128, 128))
                    qk_jtile('k', h, wts_k03, ds(h * 128, 128),
                             split=(h == 3))
            wpool_cm.__exit__(None, None, None)

            # ---------------- Phase B + interleaves ----------------
            ypool_cm = tc.tile_pool(name="ysb", bufs=1)
            ypool = ypool_cm.__enter__()
            y_t = [ypool.tile([128, T], bf16, tag=f"y{h}", name=f"y{h}")
                   for h in range(HG)]
            # reserve-weight piece pool (right side, small)
            wrpool_cm = tc.tile_pool(name="wres", bufs=1, side="right")
            wrpool = wrpool_cm.__enter__()

            espool_cm = tc.tile_pool(name="es", bufs=4)
            espool = espool_cm.__enter__()
            accpool_cm = tc.tile_pool(name="acc", bufs=2)
            accpool = accpool_cm.__enter__()
            npool_cm = tc.tile_pool(name="nrm", bufs=2)
            npool = npool_cm.__enter__()
            pss_cm = tc.tile_pool(name="psS", bufs=3, space="PSUM")
            pss = pss_cm.__enter__()
            psy_cm = tc.tile_pool(name="psY", bufs=2, space="PSUM")
            psy = psy_cm.__enter__()
            psd_cm = tc.tile_pool(name="psD", bufs=1, space="PSUM")
            psd = psd_cm.__enter__()
            aqk_cm = tc.tile_pool(name="psR", bufs=2, space="PSUM")
            aqkps = aqk_cm.__enter__()
            psoB_cm = [None]
            psoB = [None]

            wo_t = [None] * HG
            ospool = [None]   # allocated after x releases

            # --- reserve QK backfill thunks -------------------------------
            def reserve_thunks(head):
                """PE thunk list for this head's reserve j-tiles; DMAs are
                emitted immediately (they land well before use)."""
                thunks = []
                for qk, h2, tchs in RESERVE.get(head, ()):
                    jcol = h2 * 128 if qk == 'q' else JQ + h2 * 128
                    wr = []
                    for ci in range(NCT):
                        wr_ = wrpool.tile([128, 128], bf16, tag=f"wr{ci}",
                                          bufs=1, name=f"wr_{qk}{h2}_{ci}")
                        nc.sync.dma_start(
                            wr_[:], WqkT[ts(ci, 128), ds(jcol, 128)])
                        wr.append(wr_)
                    for tch in tchs:
                        def mk_chain(qk=qk, h2=h2, tch=tch, wr=wr):
                            ps_r = aqkps.tile([128, 512], f32, tag="pr",
                                              name="pr")
                            def mm(ci, ps_r=ps_r, qk=qk, h2=h2, tch=tch,
                                   wr=wr):
                                nc.tensor.matmul(
                                    ps_r[:], wr[ci][:],
                                    xt[ci][:, ts(tch, 512)],
                                    start=(ci == 0), stop=(ci == NCT - 1))
                                if ci == NCT - 1:
                                    if qk == 'q':
                                        nc.scalar.activation(
                                            QK_t[h2][:, ts(tch, 512)],
                                            ps_r[:], Ident,
                                            bias=bq_t[h2][:, 0:1])
                                    else:
                                        nc.vector.tensor_copy(
                                            QK_t[8 + h2][:, ts(tch, 512)],
                                            ps_r[:])
                            return [lambda ci=ci: mm(ci)
                                    for ci in range(NCT)]
                        thunks.append(mk_chain)
                # flatten lazily: list of chain-makers -> iterator of MMs
                def gen():
                    for mk in thunks:
                        for fn in mk():
                            yield fn
                return gen()

            # --- phase C chains -------------------------------------------
            c_emitted = [0]     # count of (tt, cch) chains already emitted

            def c_chain(idx, pool):
                tt, cch = divmod(idx, NTC)
                po = pool.tile([128, 512], f32, tag="po", name=f"po{idx}")
                def mm(ch, po=po, tt=tt, cch=cch, idx=idx):
                    nc.tensor.matmul(
                        po[:], y_t[ch][:, ts(tt, 128)],
                        wo_t[ch][:, ds(cch * 512, 512)],
                        start=(ch == 0), stop=(ch == HG - 1))
                    if ch == HG - 1:
                        st = ospool[0].tile([128, 512], bf16, tag="ost")
                        if idx % 2 == 0:
                            nc.vector.tensor_copy(st[:], po[:])
                        else:
                            nc.scalar.copy(st[:], po[:])
                        nc.sync.dma_start(
                            o[ts(tt, 128), ds(cch * 512, 512)], st[:])
                return [lambda ch=ch: mm(ch) for ch in range(HG)]

            def c_thunks(max_chains):
                def gen():
                    n = 0
                    while n < max_chains and c_emitted[0] < NTT * NTC:
                        idx = c_emitted[0]
                        c_emitted[0] += 1
                        n += 1
                        for fn in c_chain(idx, psoB[0]):
                            yield fn
                return gen()

            # --- phase B slot pipeline ------------------------------------
            LAG = 3
            for bi, h in enumerate(B_ORDER):
                if bi == 0:
                    # fill the A-QK->B transition (first scores matmuls wait
                    # on the final projection copies via psum-bank reuse)
                    # with warm-up dummies on the dep-free psR banks
                    for _ in range(6):
                        wp2 = aqkps.tile([128, 512], f32, tag="pr",
                                         name="pr")
                        nc.tensor.matmul(wp2[:], wsrc[:, 0:128], wsrc[:],
                                         start=True, stop=True)
                if h == 4:
                    # head 4 is last: backfill with early phase-C chains
                    # (chunk-0 y of all heads is complete after chunk 0)
                    backfill = None     # armed per-chunk below
                else:
                    backfill = reserve_thunks(h)
                kt = QK_t[8 + h]
                avq = []        # (due_slot, es_ap, psY_ap, v_ap, start, stop)
                events = {}     # slot -> [thunk]
                credit = [0.0]
                if backfill is not None:
                    n_res = sum(len(e[2]) * NCT for e in RESERVE.get(h, ()))
                    rate = n_res / 38.0 + 0.1
                else:
                    rate = 1.2
                backref = [backfill]

                def at_slot(s, fn):
                    events.setdefault(s, []).append(fn)

                def pump(slot):
                    for fn in events.pop(slot, []):
                        fn()
                    while avq and avq[0][0] <= slot:
                        _, e_, py_, vs_, fi_, la_ = avq.pop(0)
                        nc.tensor.matmul(py_, vs_, e_, start=fi_,
                                         stop=la_)
                    credit[0] += rate
                    while credit[0] >= 1.0 and backref[0] is not None:
                        fn = next(backref[0], None)
                        if fn is None:
                            backref[0] = None
                            break
                        fn()
                        credit[0] -= 1.0

                def mk_dens(acc_, st):
                    def fn():
                        ps_d = psd.tile([128, 512], f32, tag="pd", name="pd")
                        nc.tensor.matmul(ps_d[:], ones_sq, acc_[:],
                                         start=True, stop=True)
                        st["ps_d"] = ps_d
                    return fn

                def mk_rec(st):
                    def fn():
                        rb = npool.tile([128, 512], f32, tag="rb", name="rb")
                        nc.vector.reciprocal_approx_fast(rb[:], st["ps_d"][:])
                        st["rb"] = rb
                    return fn

                def mk_mult(ps_y_, h_, ci_, st):
                    def fn():
                        nc.vector.tensor_tensor(
                            y_t[h_][:, ds(ci_ * 512, 512)], ps_y_[:],
                            st["rb"][:], mult)
                    return fn

                slot = 0
                for ci in range(NTC):
                    ps_y = psy.tile([128, 512], f32, tag="py")
                    acc = accpool.tile([128, 512], f16, tag="acc", name="acc")
                    njt = 4 * (ci + 1)
                    # order: a full-width tile first (jt=4ci, off=0), then
                    # spread the remaining diagonal tiles (短 exp chains)
                    # through the plain-tile interior
                    plain = list(range(4 * ci))
                    diag = list(range(4 * ci + 1, njt))
                    order = [4 * ci]
                    if plain:
                        step = max(1, len(plain) // 3)
                        pi = 0
                        for dtile in diag:
                            order.extend(plain[pi:pi + step])
                            pi += step
                            order.append(dtile)
                        order.extend(plain[pi:])
                    else:
                        order.extend(diag)
                    for oi_, jt in enumerate(order):
                        off = jt * 128 - ci * 512
                        w = 512 - max(off, 0)
                        ps_s = pss.tile([128, 512], f32, tag="pss")
                        nc.tensor.matmul(
                            ps_s[:, ds(512 - w, w)], kt[:, ts(jt, 128)],
                            QK_t[h][:, ds(ci * 512 + 512 - w, w)],
                            start=True, stop=True)
                        pump(slot)
                        es = espool.tile([128, 512], f16, tag="es")
                        nc.scalar.activation(es[:, ds(512 - w, w)],
                                             ps_s[:, ds(512 - w, w)], Exp)
                        if off >= 0:
                            nc.vector.tensor_tensor(
                                es[:, ds(off, 128)], es[:, ds(off, 128)],
                                tri[:, 0:128], mult)
                        if oi_ == 0:
                            nc.vector.tensor_copy(acc[:], es[:])
                        else:
                            nc.vector.tensor_tensor(
                                acc[:, ds(512 - w, w)], acc[:, ds(512 - w, w)],
                                es[:, ds(512 - w, w)], add)
                        avq.append((slot + LAG, es[:, ds(512 - w, w)],
                                    ps_y[:, ds(512 - w, w)],
                                    V_t[jt][:, ds(h * 128, 128)],
                                    oi_ == 0, oi_ == njt - 1))
                        slot += 1
                    st = {}
                    at_slot(slot + 1, mk_dens(acc, st))
                    at_slot(slot + 2, mk_rec(st))
                    at_slot(slot + 4, mk_mult(ps_y, h, ci, st))
                    if h == 4:
                        # arm phase-C backfill once chunk-0 y is final
                        def arm():
                            if backref[0] is None:
                                credit[0] = 0.0
                                backref[0] = c_thunks(6)
                        if ci == 0:
                            at_slot(slot + 5, arm)
                # drain the pipeline and flush any leftover backfill
                while avq or events:
                    pump(slot)
                    slot += 1
                if backref[0] is not None:
                    for fn in backref[0]:
                        fn()
                    backref[0] = None

                if bi == len(B_ORDER) - 2:
                    # all reserve QK done (after B(5)): release x + wres,
                    # load the out-projection weights into the freed space
                    wrpool_cm.__exit__(None, None, None)
                    xpool_cm.__exit__(None, None, None)
                    aqk_cm.__exit__(None, None, None)
                    psoB_cm[0] = tc.tile_pool(name="psoB", bufs=2,
                                              space="PSUM")
                    psoB[0] = psoB_cm[0].__enter__()
                    wopool_cm = tc.tile_pool(name="woc", bufs=1,
                                             side="right")
                    wopool = wopool_cm.__enter__()
                    for ch in range(HG):
                        w_ = wopool.tile([128, C], bf16, tag=f"wo{ch}",
                                         name=f"wo{ch}")
                        nc.sync.dma_start(w_[:], WoT[ts(ch, 128), :])
                        wo_t[ch] = w_
                    osp_cm = tc.tile_pool(name="ost", bufs=6)
                    ospool[0] = osp_cm.__enter__()

            # ---------------- Phase C (remaining) ----------------
            psoB_cm[0].__exit__(None, None, None)
            psd_cm.__exit__(None, None, None)
            psy_cm.__exit__(None, None, None)
            pss_cm.__exit__(None, None, None)
            with tc.tile_pool(name="psO", bufs=4, space="PSUM") as psoC:
                while c_emitted[0] < NTT * NTC:
                    idx = c_emitted[0]
                    c_emitted[0] += 1
                    for fn in c_chain(idx, psoC):
                        fn()
            # release in LIFO order
            osp_cm.__exit__(None, None, None)
            wopool_cm.__exit__(None, None, None)
            npool_cm.__exit__(None, None, None)
            accpool_cm.__exit__(None, None, None)
            espool_cm.__exit__(None, None, None)
            ypool_cm.__exit__(None, None, None)
            qkpool_cm.__exit__(None, None, None)
            vpool_cm.__exit__(None, None, None)
    nc.compile()
    return nc


def _rope_matrix():
    inv_freq = 1.0 / (ROPE_BASE ** (np.arange(0, D, 2, dtype=np.float64) / D))
    freqs = np.float64(T) * inv_freq
    emb = np.concatenate([freqs, freqs])
    cos, sin = np.cos(emb), np.sin(emb)
    R = np.zeros((D, D))
    for j in range(D):
        R[j, j] = cos[j]
    for j in range(64):
        R[j, 2 * j + 1] += -sin[j]
    for j in range(64, 128):
        R[j, 2 * (j - 64)] += sin[j]
    return R


def _host_mask():
    m = np.zeros((128, 256), dtype=np.float16)
    jj = np.arange(128)
    m[:, 0:128] = (jj[:, None] <= jj[None, :])
    m[:, 128:256] = 1.0
    return m


def kernel(x, Wqkv, bqkv, Wout, bout):
    import ml_dtypes
    from concourse.bass_utils import run_bass_kernel_spmd

    bfloat16 = ml_dtypes.bfloat16

    if "nc" not in _CACHE:
        _CACHE["nc"] = _build_nc()
    nc = _CACHE["nc"]

    x = np.asarray(x, dtype=np.float32)
    Wqkv64 = np.asarray(Wqkv, dtype=np.float64)
    bqkv64 = np.asarray(bqkv, dtype=np.float64)
    Wout64 = np.asarray(Wout, dtype=np.float64)
    bout64 = np.asarray(bout, dtype=np.float64)

    R = _rope_matrix()
    scale = 1.0 / np.sqrt(np.float64(D))
    Wq = Wqkv64[:C].reshape(H, D, C)
    Wk = Wqkv64[C:2 * C].reshape(H, D, C)
    Wv = Wqkv64[2 * C:].reshape(H, D, C)
    bqv = bqkv64[:C].reshape(H, D)
    bv = bqkv64[2 * C:]

    Wq_f = np.einsum('jk,hkc->hjc', R, Wq) * scale
    bq_f = np.einsum('jk,hk->hj', R, bqv) * scale
    Wk_f = np.einsum('jk,hkc->hjc', R, Wk)
    bias_final = (bout64 + Wout64 @ bv).astype(np.float32)

    mask = _host_mask()
    in_maps = []
    xTb = [np.ascontiguousarray(x[b].T).astype(bfloat16) for b in range(B)]
    shard = {}
    for hg in range(2):
        hs = slice(hg * HG, (hg + 1) * HG)
        wqk = np.concatenate(
            [Wq_f[hs].reshape(JQ, C), Wk_f[hs].reshape(JQ, C)], axis=0)
        shard[hg] = dict(
            WqkT=np.ascontiguousarray(wqk.T).astype(bfloat16),
            WvT=np.ascontiguousarray(
                Wv[hs].reshape(JQ, C).T).astype(bfloat16),
            WoT=np.ascontiguousarray(
                Wout64[:, hg * JQ:(hg + 1) * JQ].T).astype(bfloat16),
            bq=bq_f[hs].reshape(JQ, 1).astype(np.float32),
        )
    for core in range(NCORES):
        b, hg = core // 2, core % 2
        in_maps.append(dict(xT=xTb[b], msk=mask, **shard[hg]))

    res = run_bass_kernel_spmd(nc, in_maps, core_ids=list(range(NCORES)),
                               **_CACHE.get("run_kwargs", {}))
    _CACHE["last_result"] = res
    out = np.empty((B, T, C), dtype=np.float32)
    for b in range(B):
        out[b] = (res.results[2 * b]["o"].astype(np.float32)
                  + res.results[2 * b + 1]["o"].astype(np.float32)
                  + bias_final)
    return out
